# revision 52
# baseline (speedup 1.0000x reference)
"""Trainium2 Bass kernel for a 2-layer GCN (GCNConv+BN+ReLU, GCNConv+BN).

Self-contained: kernel(**inputs) takes the full unsharded inputs, shards
destinations across 8 NeuronCores (edges partitioned by destination, one-hot
matmul scatter-add on the TensorEngine, dma_gather message fetch from fp16
feature tables rotated across 4 SWDGE queues, piece-wise AllGather for
tables, AllReduce for BN stats), and returns the full [100000, 128] float32
output.
"""
import numpy as np


def _apply_tile_patch():
    """This walrus build allows only 1 sync wait per CTRL instruction and
    rejects long semaphore range clears; spread Tile's exit-drain waits
    across nops and chunk the sem recycles."""
    import concourse.tile as tile_mod
    import concourse.mybir as mybir
    from concourse.vector_clock import ScopedClock

    MAXW = 1

    def _patched_drain_and_barrier(self, tick_clock, wait_clock):
        nc = self.nc
        tmp = nc.sync.nop(nofuse=True, hint="drain_wait_spread")
        wait_clock.add_sem_waits(tmp.ins, ScopedClock({None: tick_clock.global_clock}))
        waits = list(tmp.ins.sync_info.on_wait or [])
        if len(waits) > MAXW:
            tmp.ins.sync_info = mybir.SyncInfo(on_wait=waits[:MAXW], on_update=[])
            for i in range(MAXW, len(waits), MAXW):
                extra = nc.sync.nop(nofuse=True, hint="drain_wait_spread")
                extra.ins.sync_info = mybir.SyncInfo(
                    on_wait=waits[i:i + MAXW], on_update=[])
        nc.sync.drain()
        nc.all_engine_barrier()
        assert self.sems is not None
        popped = nc._tile_sem_poison_stack.pop()
        assert popped is self._sem_poison
        sems = list(self.sems.allocated().values())
        for i in range(0, len(sems), 8):
            nc.clear_and_free_semaphores(sems[i:i + 8])
        nc.all_engine_barrier()

    tile_mod.TileContext._drain_and_barrier = _patched_drain_and_barrier


from dataclasses import dataclass, field


@dataclass
class Cfg:
    N: int = 100000
    D: int = 128
    CORES: int = 8
    WIN: int = 128
    SBW: int = 6          # windows per superblock (PSUM-resident agg tiles)
    SEG: int = 25000      # gather table segment rows (int16 index reach)
    BN_EPS: float = 1e-5
    NQ: int = 4           # SWDGE queues for gather desc-gen rotation
    GSPLIT: int = 8       # max chunks per gather sub-call (0 = whole call);
                          # 8 chunks = 1024 descs = one SWDGE ring side (the
                          # device ucode cannot take a bigger single call)
    DMASCRATCH: int = 16384  # SWDGE ring carveout (bytes/partition)
    BN_AG: bool = True    # BN stats via AllGather+local reduce (vs AllReduce)
    BATCH_IO: bool = True  # 4-tile batched phase-A/h2/y DMAs
    SEGOUTER: bool = False  # seg-outer call order (overlaps table AllGather)
                           # vs sb-outer (PSUM accumulates across segments)

    def __post_init__(self):
        import os
        if os.environ.get("GNN_NQ"):
            self.NQ = int(os.environ["GNN_NQ"])
        if os.environ.get("GNN_GSPLIT") is not None and os.environ.get("GNN_GSPLIT") != "":
            self.GSPLIT = int(os.environ["GNN_GSPLIT"])
        if os.environ.get("GNN_BNAG"):
            self.BN_AG = os.environ["GNN_BNAG"] == "1"
        if os.environ.get("GNN_BATCHIO"):
            self.BATCH_IO = os.environ["GNN_BATCHIO"] == "1"
        if os.environ.get("GNN_SEGOUTER"):
            self.SEGOUTER = os.environ["GNN_SEGOUTER"] == "1"

    @property
    def PIECE(self):
        # per-core rows contributed to one pipelined AllGather piece
        return self.SEG // self.CORES

    @property
    def NPC(self):
        return self.N // self.CORES

    @property
    def NW(self):
        return (self.NPC + self.WIN - 1) // self.WIN

    @property
    def NSB(self):
        return (self.NW + self.SBW - 1) // self.SBW

    @property
    def NSEG(self):
        return (self.N + self.SEG - 1) // self.SEG


@dataclass
class Sched:
    # slots[(sb, seg)] = list of (w_global, j_ordinal); chunk columns laid out
    # in this order (j-major), concatenated over (sb, seg).
    slots: dict = field(default_factory=dict)
    chunk_off: dict = field(default_factory=dict)   # (sb, seg) -> first chunk col
    n_chunks_call: dict = field(default_factory=dict)  # (sb, seg) -> chunks in call
    CT: int = 0                                     # total chunk columns
    win_total: dict = field(default_factory=dict)   # w -> total chunks (all segs)


def make_schedule(counts_max, cfg: Cfg) -> Sched:
    """counts_max[w, s] = max edge count over cores for (window w, segment s).

    Slots within one (sb, seg) call are ordered j-major (all windows' chunk 0
    first, then chunk 1, ...) so per-core trailing padding is maximal.
    """
    s = Sched()
    ct = 0
    for sb in range(cfg.NSB):
        w0, w1 = sb * cfg.SBW, min((sb + 1) * cfg.SBW, cfg.NW)
        for seg in range(cfg.NSEG):
            nch = {w: int(-(-counts_max[w, seg] // 128)) for w in range(w0, w1)}
            order = []
            j = 0
            while True:
                row = [(w, j) for w in range(w0, w1) if nch[w] > j]
                if not row:
                    break
                order.extend(row)
                j += 1
            for (w, _) in order:
                s.win_total[w] = s.win_total.get(w, 0) + 1
            s.slots[(sb, seg)] = order
            s.chunk_off[(sb, seg)] = ct
            s.n_chunks_call[(sb, seg)] = len(order)
            ct += len(order)
    s.CT = ct
    return s


def prep(edge_index: np.ndarray, edge_weight: np.ndarray, cfg: Cfg):
    """Host preprocessing: normalization, dest-sharding, chunk packing.

    Returns (sched, per_core) where per_core[i] has meta [128, 8*CT] int16
    and ohmat [128, 128*(CT+NW)] f16 (scatter one-hots + self-loop diags).
    """
    N = cfg.N
    row = np.concatenate([edge_index[0], np.arange(N, dtype=np.int64)]).astype(np.int64)
    col = np.concatenate([edge_index[1], np.arange(N, dtype=np.int64)]).astype(np.int64)
    w = np.concatenate([edge_weight.astype(np.float64), np.ones(N)])

    deg = np.bincount(col, weights=w, minlength=N)
    dinv = np.where(deg > 0, 1.0 / np.sqrt(np.maximum(deg, 1e-12)), 0.0)
    norm_all = (dinv[row] * w * dinv[col]).astype(np.float32)

    # the appended unit-weight self-loops bypass the gather entirely: their
    # message rows are the core-local h tile, streamed node-major.
    nsl = norm_all[len(edge_index[0]):]           # [N], self-loop norms
    row = row[:len(edge_index[0])]
    col = col[:len(edge_index[0])]
    norm = norm_all[:len(edge_index[0])]

    core = col // cfg.NPC
    d = col % cfg.NPC
    win = d // cfg.WIN
    slot = d % cfg.WIN
    # table row of node n (piece-wise AllGather layout):
    #   c = n // NPC, r = n % NPC, q = r // PIECE
    #   trow = q * SEG + c * PIECE + r % PIECE ; seg = q
    src_c = row // cfg.NPC
    src_r = row % cfg.NPC
    seg = src_r // cfg.PIECE
    srcid = (src_c * cfg.PIECE + src_r % cfg.PIECE).astype(np.int16)

    # group id per edge: (core, sb, seg, win)
    sb = win // cfg.SBW
    NW, NSEG = cfg.NW, cfg.NSEG
    gid = ((core * cfg.NSB + sb) * NSEG + seg) * NW + win
    order = np.argsort(gid, kind="stable")
    gid_s = gid[order]
    srcid_s = srcid[order]
    slot_s = slot[order].astype(np.float32)
    norm_s = norm[order]

    # counts per (core, win, seg)
    n_gids = cfg.CORES * cfg.NSB * NSEG * NW
    cnt = np.bincount(gid, minlength=n_gids)
    cntr = cnt.reshape(cfg.CORES, cfg.NSB, NSEG, NW)
    w_idx = np.arange(NW)
    # counts[c, w, s]: only the (sb = w // SBW) plane is populated
    counts = cntr[:, w_idx // cfg.SBW, :, w_idx]      # [NW, CORES, NSEG]
    counts = counts.transpose(1, 0, 2)                # [CORES, NW, NSEG]
    counts_max = counts.max(axis=0)  # [NW, NSEG]
    gsp = cfg.GSPLIT if cfg.GSPLIT else 10 ** 9

    sched = make_schedule(counts_max, cfg)
    CT = sched.CT

    starts = np.zeros(n_gids + 1, dtype=np.int64)
    np.cumsum(cnt, out=starts[1:])

    per_core = []
    for ci in range(cfg.CORES):
        src_p = np.zeros((CT, 128), dtype=np.int16)
        slot_p = np.zeros((CT, 128), dtype=np.int32)
        val_p = np.zeros((CT, 128), dtype=np.float32)
        for sbi in range(cfg.NSB):
            for sg in range(NSEG):
                base = sched.chunk_off[(sbi, sg)]
                for k, (wg, j) in enumerate(sched.slots[(sbi, sg)]):
                    g = ((ci * cfg.NSB + sbi) * NSEG + sg) * NW + wg
                    a, b = starts[g], starts[g + 1]
                    lo = a + j * 128
                    hi = min(a + (j + 1) * 128, b)
                    r = max(0, hi - lo)
                    if r:
                        flat0 = (base + k) * 128
                        src_p.reshape(-1)[flat0:flat0 + r] = srcid_s[lo:hi]
                        slot_p.reshape(-1)[flat0:flat0 + r] = slot_s[lo:hi]
                        val_p.reshape(-1)[flat0:flat0 + r] = norm_s[lo:hi]
        # per-core trim: fully-padded trailing chunks of each gather sub-call
        # are marked idx=-1 and skipped at runtime via num_idxs_reg (ccnt).
        # Keep >=1 chunk per sub-call so every SDMA engine still gets descs
        # (the completion sem needs all 16 engines to tick).
        src_i32 = src_p.astype(np.int32)
        ccnt = []
        for sbi in range(cfg.NSB):
            for sg in range(NSEG):
                nch = sched.n_chunks_call[(sbi, sg)]
                if not nch:
                    continue
                o = sched.chunk_off[(sbi, sg)]
                kept = [
                    j * 128 < counts[ci, wg, sg]
                    for (wg, j) in sched.slots[(sbi, sg)]]
                real = [
                    int(min(max(counts[ci, wg, sg] - j * 128, 0), 128))
                    for (wg, j) in sched.slots[(sbi, sg)]]
                for c0 in range(0, nch, gsp):
                    c1 = min(c0 + gsp, nch)
                    l = c0
                    for k in range(c0, c1):
                        if kept[k]:
                            l = k
                    # desc-granular: trim the final kept chunk's tail too,
                    # but never below 128 descs (all 16 SDMA engines must
                    # tick the completion sem).
                    tpos = max((l - c0) * 128 + max(real[l], 1), 128)
                    for k in range(l + 1, c1):
                        src_i32[o + k, :] = -1
                    lrow = c0 + (tpos - 1) // 128   # last chunk with valid descs
                    r = tpos - (lrow - c0) * 128    # valid descs in that chunk
                    if r < 128:
                        src_i32[o + lrow, r:] = -1
                    ccnt.append(tpos)
        # meta [128, 8*CT] int16: per (sb,seg) call gather idxs at col 8*off,
        # flat idx j -> [j % 16, j // 16]
        meta = np.zeros((128, 8 * CT), dtype=np.int16)
        for sbi in range(cfg.NSB):
            for sg in range(NSEG):
                nch = sched.n_chunks_call[(sbi, sg)]
                if not nch:
                    continue
                o = sched.chunk_off[(sbi, sg)]
                flat = src_i32.astype(np.int16).reshape(-1)[
                    o * 128:(o + nch) * 128]
                wrapped = flat.reshape(-1, 16).T  # [16, nch*8]
                meta[:, o * 8:(o + nch) * 8] = np.tile(wrapped, (8, 1))
        # host-built scatter one-hot tiles [edge p, slot j] per chunk, scaled
        # by norm; padding rows are all-zero so garbage gathers cancel.
        # NW extra diag tiles at the end carry the self-loop norms.
        oh = np.zeros((CT + cfg.NW, 128, 128), dtype=np.float16)
        oh[np.arange(CT)[:, None], np.arange(128)[None, :], slot_p] = (
            val_p.astype(np.float16))
        nslc = nsl[ci * cfg.NPC:(ci + 1) * cfg.NPC]
        for wgi in range(cfg.NW):
            seg_v = nslc[wgi * 128:(wgi + 1) * 128].astype(np.float16)
            np.fill_diagonal(oh[CT + wgi, :len(seg_v), :len(seg_v)], seg_v)
        ohmat = np.ascontiguousarray(oh.transpose(1, 0, 2).reshape(128, -1))
        per_core.append({"meta": meta, "ohmat": ohmat,
                         "ccnt": np.asarray(ccnt, np.int32).reshape(1, -1)})
    # sub-calls where no core trims keep their compile-time count (skips the
    # per-sub-call reg_load on the Pool engine, which paces the edge phase)
    full = []
    for sbi in range(cfg.NSB):
        for sg in range(NSEG):
            nch = sched.n_chunks_call[(sbi, sg)]
            if not nch:
                continue
            for c0 in range(0, nch, gsp):
                full.append((min(c0 + gsp, nch) - c0) * 128)
    full = np.asarray(full, np.int32)
    sched.anytrim = (np.stack(
        [pc["ccnt"][0] for pc in per_core]).min(axis=0) < full).tolist()
    return sched, per_core


def build(nc, tc, cfg: Cfg, sched: Sched, tensors):
    """Emit the kernel into TileContext tc. tensors: dict of dram handles."""
    import concourse.mybir as mybir
    from concourse.bass import ts as _ts  # noqa: F401

    f32 = mybir.dt.float32
    f16 = mybir.dt.float16
    TDT = f16
    i16 = mybir.dt.int16
    i32 = mybir.dt.int32
    Alu = mybir.AluOpType
    Act = mybir.ActivationFunctionType

    N, D, NPC, WIN, NW, NSB, SBW = (
        cfg.N, cfg.D, cfg.NPC, cfg.WIN, cfg.NW, cfg.NSB, cfg.SBW)
    NSEG, SEG = cfg.NSEG, cfg.SEG
    NWPAD = NW * WIN
    NCALLS = NSB * NSEG
    CT = sched.CT

    x = tensors["x_shard"]
    meta = tensors["meta"]
    ohmat = tensors["ohmat"]
    W1, W2 = tensors["W1"], tensors["W2"]
    bias = {1: tensors["b1"], 2: tensors["b2"]}
    gam = {1: tensors["g1"], 2: tensors["g2"]}
    bet = {1: tensors["be1"], 2: tensors["be2"]}
    y = tensors["y"]

    CHMAX = max(sched.n_chunks_call.values())
    SBCH = max(sum(sched.n_chunks_call[(sb, sg)] for sg in range(cfg.NSEG))
               for sb in range(cfg.NSB))

    from contextlib import ExitStack
    es = tc._gnn_exitstack = ExitStack()
    const = es.enter_context(tc.tile_pool(name="const", bufs=1))
    zpool = es.enter_context(tc.tile_pool(name="zres", bufs=1))
    spool = es.enter_context(tc.tile_pool(name="stats", bufs=1))
    work = es.enter_context(tc.tile_pool(name="work", bufs=3))
    ohp = es.enter_context(tc.tile_pool(name="oh", bufs=6))
    msgp = es.enter_context(tc.tile_pool(name="msg", bufs=7))
    slp = es.enter_context(tc.tile_pool(name="slmsg", bufs=4))
    idxp = es.enter_context(tc.tile_pool(name="idx", bufs=5))
    scr = es.enter_context(tc.tile_pool(name="scr", bufs=2))
    colp = es.enter_context(tc.tile_pool(name="col", bufs=1))
    psum_agg = es.enter_context(tc.tile_pool(
        name="pagg", bufs=(2 * SBW if cfg.SEGOUTER else SBW), space="PSUM"))
    psum_misc = es.enter_context(tc.tile_pool(name="pmisc", bufs=2, space="PSUM"))
    dram = es.enter_context(tc.tile_pool(name="dram", bufs=1, space="DRAM"))

    # ---- constants (iota rows/col provided by host via "consts" input) ----
    from concourse import library_config
    from concourse.bass import _add_dep_helper
    lib_inst = nc.gpsimd.load_library(library_config.mlp)
    consts_t = const.tile([128, 129], f32, name="consts_t")
    nc.sync.dma_start(consts_t[:], tensors["consts"][:, :])
    iota_row = consts_t[:, 0:128]
    iota_col = consts_t[:, 128:129]
    ident = const.tile([128, 128], f32)
    nc.vector.tensor_scalar(
        out=ident[:], in0=iota_row, scalar1=iota_col, scalar2=None,
        op0=Alu.is_equal)

    _nreg_cache = {}

    def nidx_reg(v):
        if v not in _nreg_cache:
            r = nc.gpsimd.alloc_register(f"nidx_{v}")
            nc.gpsimd.reg_mov(r, v)
            _nreg_cache[v] = r
        return _nreg_cache[v]

    # per-(core, sub-call) runtime desc counts: prep marks each core's
    # fully-padded trailing chunks idx=-1; the gather skips them when
    # num_idxs_reg holds the per-core valid count.
    gsp = cfg.GSPLIT if cfg.GSPLIT else 10 ** 9
    subidx = {}
    nsub = 0
    for _sb in range(NSB):
        for _sg in range(NSEG):
            _n = sched.n_chunks_call[(_sb, _sg)]
            if not _n:
                continue
            for _c0 in range(0, _n, gsp):
                subidx[(_sb, _sg, _c0)] = nsub
                nsub += 1
    cct = const.tile([1, nsub], i32, tag="ccnt")
    nc.sync.dma_start(cct[:, :], tensors["ccnt"][:, :])
    rtrim = nc.gpsimd.alloc_register("trim")

    # per-channel params as [128,1] columns
    cols = {}
    for nm in ("b1", "g1", "be1", "b2", "g2", "be2"):
        t = colp.tile([128, 1], f32, tag=nm)
        nc.sync.dma_start(t[:], tensors[nm][:, :])
        cols[nm] = t

    # ---- DRAM scratch ----
    # local h shards split per AllGather piece so each piece's collective
    # depends only on its own rows (whole-tensor dep tracking otherwise
    # serializes the AllGather behind the entire phase).
    PIECE = cfg.PIECE
    h_local = [dram.tile([PIECE, D], TDT, tag=f"h_local{q}",
                         name=f"h_local{q}") for q in range(NSEG)]
    sh = "Local"
    h1_full = nc.dram_tensor("h1_full", [N, D], TDT, kind="Internal",
                             addr_space=sh)
    h2_full = nc.dram_tensor("h2_full", [N, D], TDT, kind="Internal",
                             addr_space=sh)
    h2_local = [dram.tile([PIECE, D], TDT, tag=f"h2_local{q}",
                          name=f"h2_local{q}") for q in range(NSEG)]

    def piece_split(g0, p):
        """(piece, local_r0, local_r1, dst_off) runs covering local node rows
        [g0, g0+p)."""
        res = []
        g1 = g0 + p
        off = 0
        while g0 < g1:
            q = g0 // PIECE
            r0 = g0 - q * PIECE
            r1 = min(r0 + (g1 - g0), PIECE)
            res.append((q, r0, r1, off))
            off += r1 - r0
            g0 += r1 - r0
        return res

    def h_read(h_loc, dst, dcol, g0, p):
        for (q, r0, r1, d0) in piece_split(g0, p):
            nc.sync.dma_start(dst[d0:d0 + (r1 - r0), dcol:dcol + 128],
                              h_loc[q][r0:r1, :])

    def h_write(h_loc, src, scol, g0, p):
        for (q, r0, r1, d0) in piece_split(g0, p):
            nc.sync.dma_start(h_loc[q][r0:r1, :],
                              src[d0:d0 + (r1 - r0), scol:scol + 128])
    bn_in = dram.tile([128, 2], f32, tag="bn_in")
    if cfg.BN_AG:
        bn_out = nc.dram_tensor("bn_gath", [2, 128 * cfg.CORES, 2], f32,
                                kind="Internal", addr_space="Local")
    else:
        bn_out = dram.tile([2, 128, 2], f32, tag="bn_out")

    import os as _os0
    if _os0.environ.get("GNN_STAGE") == "w":
        return
    # ---- phase A: h1_local = x_shard @ W1, node-major ----
    w1t = const.tile([128, 128], f32, tag="w1")
    nc.sync.dma_start(w1t[:], W1[:, :])
    w2t = const.tile([128, 128], f32, tag="w2")
    nc.sync.dma_start(w2t[:], W2[:, :])
    w2h = const.tile([128, 128], TDT, tag="w2h")
    nc.vector.tensor_copy(out=w2h[:], in_=w2t[:])

    # pre-zero msg buffers: REGTRIM leaves trailing columns stale; stale SBUF
    # garbage can decode as inf/nan and 0*inf=NaN would poison the PSUM accum.
    msg_tiles = []
    for i in range(7):
        mz = msgp.tile([128, CHMAX * 128], TDT, tag="msg")
        nc.vector.memset(mz[:], 0.0)
        msg_tiles.append(mz)

    def phase_a_tile(xsrc, p, hb, hoff):
        xT = psum_misc.tile([128, 512], f32, tag="pm")
        nc.tensor.transpose(xT[:, :p], xsrc, ident[:p, :p])
        xTs = work.tile([128, 128], f32, tag="xts")
        nc.vector.tensor_copy(out=xTs[:, :p], in_=xT[:, :p])
        hp = psum_misc.tile([128, 512], f32, tag="pm")
        nc.tensor.matmul(hp[:p, :128], lhsT=xTs[:, :p], rhs=w1t[:],
                         start=True, stop=True)
        nc.vector.tensor_copy(out=hb[:p, hoff:hoff + 128], in_=hp[:p, :128])

    for q in range(NSEG):
        qbase = q * PIECE
        PFULL = (PIECE // 512) * 512
        for b0 in range(0, PFULL, 512):
            nt = 4
            xb = work.tile([128, 4 * 128], f32, tag="xt")
            nc.sync.dma_start(
                xb[:, :nt * 128].rearrange("p (t d) -> p t d", d=128),
                x[qbase + b0:qbase + b0 + 512, :].rearrange(
                    "(t p) d -> p t d", p=128))
            hb = work.tile([128, 4 * 128], TDT, tag="hs")
            for ti in range(nt):
                phase_a_tile(xb[:, ti * 128:ti * 128 + 128], 128, hb, ti * 128)
            nc.sync.dma_start(
                h_local[q][b0:b0 + 512, :].rearrange("(t p) d -> p t d", p=128),
                hb[:, :nt * 128].rearrange("p (t d) -> p t d", d=128))
        t0 = PFULL
        while t0 < PIECE:
            p = min(128, PIECE - t0)
            xt = work.tile([128, 128], f32, tag="xt")
            nc.sync.dma_start(xt[:p, :], x[qbase + t0:qbase + t0 + p, :])
            hb = work.tile([128, 128], TDT, tag="hs")
            phase_a_tile(xt[:p, :], p, hb, 0)
            nc.sync.dma_start(h_local[q][t0:t0 + p, :], hb[:p, :128])
            t0 += p
        nc.gpsimd.collective_compute(
            "AllGather", Alu.bypass,
            replica_groups=[list(range(cfg.CORES))],
            ins=[h_local[q][:, :]],
            outs=[h1_full[q * SEG:(q + 1) * SEG, :]])

    if _os0.environ.get("GNN_STAGE") == "ph":
        return

    # ---- per-layer ----
    zres = zpool.tile([128, NWPAD], TDT, tag="z")
    stats1 = spool.tile([128, NW], f32, tag="s1")
    stats2 = spool.tile([128, NW], f32, tag="s2")

    call_counter = [0]

    def edge_layer(lyr, table, h_loc):
        """Aggregate msgs into zres (channel-major, + bias); fill stats.

        Self-loop pass first (streams the local h tile, no gather, no table
        dependency — overlaps the table AllGather), then one pass per table
        segment accumulating into zres via SBUF adds so each segment's work
        starts as soon as its AllGather piece lands.
        """
        b_col = cols[f"b{lyr}"]
        for sb in range(NSB):
            w0 = sb * SBW
            w1_ = min(w0 + SBW, NW)
            nwin = w1_ - w0
            wt = {w: psum_agg.tile([128, 128], f32, tag="aggw", name=f"slw{w}")
                  for w in range(w0, w1_)}
            ohd = ohp.tile([128, CHMAX * 128], TDT, tag="ohb")
            nc.scalar.dma_start(ohd[:, :nwin * 128],
                                ohmat[:, (CT + w0) * 128:(CT + w1_) * 128])
            smb = slp.tile([128, SBW * 128], TDT, tag="slm")
            for w in range(w0, w1_):
                p = min(WIN, NPC - w * WIN)
                h_read(h_loc, smb, (w - w0) * 128, w * WIN, p)
            for w in range(w0, w1_):
                p = min(WIN, NPC - w * WIN)
                j0 = (w - w0) * 128
                nc.tensor.matmul(
                    wt[w][:], lhsT=smb[:p, j0:j0 + 128],
                    rhs=ohd[:p, j0:j0 + 128], start=True, stop=True)
            for w in range(w0, w1_):
                zsl = zres[:, w * 128:w * 128 + 128]
                nc.vector.tensor_scalar(
                    out=zsl, in0=wt[w][:], scalar1=b_col[:, 0:1], scalar2=None,
                    op0=Alu.add)
        def gather_call(sb, sg, wt, win_seen, win_stop, itb=None, sb_off=0):
            """Issue gathers + scatter matmuls for call (sb, sg)."""
            nch = sched.n_chunks_call[(sb, sg)]
            if nch == 0:
                return
            off = sched.chunk_off[(sb, sg)]
            if itb is None:
                it = idxp.tile([128, 8 * CHMAX], i16, tag="it")
                nc.sync.dma_start(it[:, :8 * nch],
                                  meta[:, off * 8:(off + nch) * 8])
                ib = 0
            else:
                it = itb
                ib = off - sb_off
            ohb = ohp.tile([128, CHMAX * 128], TDT, tag="ohb")
            nc.scalar.dma_start(ohb[:, :nch * 128],
                                ohmat[:, off * 128:(off + nch) * 128])
            msg = msgp.tile([128, CHMAX * 128], TDT, tag="msg")
            step = cfg.GSPLIT if cfg.GSPLIT else nch
            for c0 in range(0, nch, step):
                c1 = min(c0 + step, nch)
                scn = c1 - c0
                # queue must rotate in lockstep with Tile's DMASW sem
                # lane rotation (one bump per Pool DMA instruction), so
                # each DMASW lane always pairs with one SWDGE queue.
                qn = call_counter[0] % cfg.NQ
                call_counter[0] += 1
                mview = msg[:, c0 * 128:c1 * 128].rearrange(
                    "p (c e) -> p c e", e=128)
                si = subidx[(sb, sg, c0)]
                if sched.anytrim[si]:
                    nc.gpsimd.reg_load(rtrim, cct[0:1, si:si + 1])
                    nreg = rtrim
                else:
                    nreg = nidx_reg(scn * 128)
                gi = nc.gpsimd.dma_gather(
                    out_ap=mview, in_ap=table[sg * SEG:(sg + 1) * SEG, :],
                    idxs_ap=it[:, 8 * (ib + c0):8 * (ib + c1)],
                    num_idxs=scn * 128,
                    num_idxs_reg=nreg, elem_size=128,
                    single_packet=True, queue_num=qn)
                _add_dep_helper(gi.ins, lib_inst.ins, sync=False,
                                reason="gpsimd library order")
            for ci, (w, j) in enumerate(sched.slots[(sb, sg)]):
                seen = win_seen.get(w, 0)
                nc.tensor.matmul(
                    wt[w][:], lhsT=msg[:, ci * 128:(ci + 1) * 128],
                    rhs=ohb[:, ci * 128:(ci + 1) * 128], start=(seen == 0),
                    stop=(seen == win_stop[w] - 1))
                win_seen[w] = seen + 1

        def win_stats(w):
            wdst = min(WIN, NPC - w * WIN)
            nc.vector.tensor_reduce(
                out=stats1[:, w:w + 1], in_=zres[:, w * 128:w * 128 + wdst],
                axis=mybir.AxisListType.X, op=Alu.add)
            sq = scr.tile([128, 128], f32, tag="sq")
            nc.scalar.activation(
                out=sq[:, :wdst], in_=zres[:, w * 128:w * 128 + wdst],
                func=Act.Square, accum_out=stats2[:, w:w + 1])

        if cfg.SEGOUTER:
            for sg in range(NSEG):
                last = sg == NSEG - 1
                for sb in range(NSB):
                    w0 = sb * SBW
                    w1_ = min(w0 + SBW, NW)
                    nch = sched.n_chunks_call[(sb, sg)]
                    if nch == 0 and not last:
                        continue
                    wt = {w: psum_agg.tile([128, 128], f32, tag="aggw",
                                           name=f"aggw{w}")
                          for w in range(w0, w1_)} if nch else {}
                    win_seen = {}
                    win_stop = {}
                    for (w, j) in sched.slots[(sb, sg)]:
                        win_stop[w] = win_stop.get(w, 0) + 1
                    gather_call(sb, sg, wt, win_seen, win_stop)
                    for w in range(w0, w1_):
                        if w not in win_seen:
                            continue
                        zsl = zres[:, w * 128:w * 128 + 128]
                        nc.vector.tensor_tensor(
                            out=zsl, in0=zsl, in1=wt[w][:], op=Alu.add)
                    if last:
                        for w in range(w0, w1_):
                            win_stats(w)
        else:
            for sb in range(NSB):
                w0 = sb * SBW
                w1_ = min(w0 + SBW, NW)
                wt = {w: psum_agg.tile([128, 128], f32, tag="aggw",
                                       name=f"aggw{w}")
                      for w in range(w0, w1_)}
                win_seen = {}
                win_stop = {w: sched.win_total.get(w, 0)
                            for w in range(w0, w1_)}
                sb_off = sched.chunk_off[(sb, 0)]
                sb_tot = sum(sched.n_chunks_call[(sb, sg)]
                             for sg in range(NSEG))
                itb = idxp.tile([128, 8 * SBCH], i16, tag="it")
                nc.sync.dma_start(itb[:, :8 * sb_tot],
                                  meta[:, sb_off * 8:(sb_off + sb_tot) * 8])
                for sg in range(NSEG):
                    gather_call(sb, sg, wt, win_seen, win_stop, itb, sb_off)
                for w in range(w0, w1_):
                    if win_seen.get(w):
                        zsl = zres[:, w * 128:w * 128 + 128]
                        nc.vector.tensor_tensor(
                            out=zsl, in0=zsl, in1=wt[w][:], op=Alu.add)
                    win_stats(w)

    def bn_reduce(lyr):
        """AllReduce stats; returns (a_col, bb_col) affine tiles."""
        s_all = scr.tile([128, 2], f32, tag="sall")
        nc.vector.tensor_reduce(out=s_all[:, 0:1], in_=stats1[:, :NW],
                                axis=mybir.AxisListType.X, op=Alu.add)
        nc.vector.tensor_reduce(out=s_all[:, 1:2], in_=stats2[:, :NW],
                                axis=mybir.AxisListType.X, op=Alu.add)
        nc.sync.dma_start(bn_in[:, :], s_all[:, :])
        if cfg.BN_AG:
            nc.gpsimd.collective_compute(
                "AllGather", Alu.bypass,
                replica_groups=[list(range(cfg.CORES))],
                ins=[bn_in[:, :]], outs=[bn_out[lyr - 1, :, :]])
            # rows g*128+p -> SBUF [p, (g s)]; sum over g on DVE
            gsb = scr.tile([128, 2 * cfg.CORES], f32, tag=f"bng{lyr}")
            gview = bn_out[lyr - 1, :, :].rearrange(
                "(g p) s -> p g s", p=128)
            nc.sync.dma_start(
                gsb[:, :].rearrange("p (g s) -> p g s", s=2), gview)
            st = colp.tile([128, 2], f32, tag=f"bnst{lyr}")
            for s in range(2):
                nc.vector.tensor_reduce(
                    out=st[:, s:s + 1], in_=gsb[:, s::2],
                    axis=mybir.AxisListType.X, op=Alu.add)
        else:
            nc.gpsimd.collective_compute(
                "AllReduce", Alu.add,
                replica_groups=[list(range(cfg.CORES))],
                ins=[bn_in[:, :]], outs=[bn_out[lyr - 1, :, :]])
            st = colp.tile([128, 2], f32, tag=f"bnst{lyr}")
            nc.sync.dma_start(st[:, :], bn_out[lyr - 1, :, :])
        mu = colp.tile([128, 1], f32, tag=f"mu{lyr}")
        nc.vector.tensor_scalar(out=mu[:], in0=st[:, 0:1], scalar1=1.0 / N,
                                scalar2=None, op0=Alu.mult)
        e2 = colp.tile([128, 1], f32, tag=f"e2{lyr}")
        nc.vector.tensor_scalar(out=e2[:], in0=st[:, 1:2], scalar1=1.0 / N,
                                scalar2=None, op0=Alu.mult)
        var = colp.tile([128, 1], f32, tag=f"var{lyr}")
        nc.vector.tensor_tensor(out=var[:], in0=mu[:], in1=mu[:], op=Alu.mult)
        nc.vector.tensor_tensor(out=var[:], in0=e2[:], in1=var[:],
                                op=Alu.subtract)
        nc.vector.tensor_scalar(out=var[:], in0=var[:], scalar1=cfg.BN_EPS,
                                scalar2=None, op0=Alu.add)
        inv = colp.tile([128, 1], f32, tag=f"inv{lyr}")
        nc.vector.reciprocal(out=inv[:], in_=var[:])
        rstd = colp.tile([128, 1], f32, tag=f"rstd{lyr}")
        nc.scalar.sqrt(out=rstd[:], in_=inv[:])
        a = colp.tile([128, 1], f32, tag=f"a{lyr}")
        nc.vector.tensor_tensor(out=a[:], in0=cols[f"g{lyr}"][:], in1=rstd[:],
                                op=Alu.mult)
        bb = colp.tile([128, 1], f32, tag=f"bb{lyr}")
        nc.vector.tensor_tensor(out=bb[:], in0=mu[:], in1=a[:], op=Alu.mult)
        nc.vector.tensor_tensor(out=bb[:], in0=cols[f"be{lyr}"][:], in1=bb[:],
                                op=Alu.subtract)
        return a, bb

    import os as _os
    _stage = _os.environ.get("GNN_STAGE", "full")
    if _stage == "a":
        return
    # ======== layer 1 ========
    edge_layer(1, h1_full, h_local)
    if _stage == "l1":
        for _ in range(int(_os.environ.get("GNN_L1_REPEAT", "1")) - 1):
            edge_layer(1, h1_full, h_local)
        return
    a1, bb1 = bn_reduce(1)
    if _stage == "bn1":
        return
    for w in range(NW):
        zsl = zres[:, w * 128:(w + 1) * 128]
        nc.scalar.activation(out=zsl, in_=zsl, func=Act.Relu,
                             scale=a1[:, 0:1], bias=bb1[:, 0:1])

    # h2_local = z1 @ W2 (z1 channel-major resident) -> node-major DRAM;
    # each AllGather piece fires as soon as its rows land.
    ag2_done = 0
    for c0 in range(0, NWPAD, 512):
        cw = min(512, NWPAD - c0)
        hp = psum_misc.tile([128, 512], f32, tag="pm")
        nc.tensor.matmul(hp[:, :cw], lhsT=w2h[:], rhs=zres[:, c0:c0 + cw],
                         start=True, stop=True)
        hsb = work.tile([128, 512], f32, tag="h2s")
        nc.vector.tensor_copy(out=hsb[:, :cw], in_=hp[:, :cw])
        sb4 = work.tile([128, 512], TDT, tag="tnmh")
        for j0 in range(0, cw, 128):
            n0 = c0 + j0
            cnt = min(128, NPC - n0)
            if cnt <= 0:
                break
            tp = psum_misc.tile([128, 512], f32, tag="pm")
            nc.tensor.transpose(tp[:, :128], hsb[:, j0:j0 + 128], ident[:])
            nc.vector.tensor_copy(out=sb4[:cnt, j0:j0 + 128],
                                  in_=tp[:cnt, :128])
            h_write(h2_local, sb4, j0, n0, cnt)
        while (ag2_done < NSEG
               and (ag2_done + 1) * PIECE <= min(c0 + cw, NPC)):
            q = ag2_done
            nc.gpsimd.collective_compute(
                "AllGather", Alu.bypass,
                replica_groups=[list(range(cfg.CORES))],
                ins=[h2_local[q][:, :]],
                outs=[h2_full[q * SEG:(q + 1) * SEG, :]])
            ag2_done += 1

    if _stage == "h2":
        return
    # ======== layer 2 ========
    edge_layer(2, h2_full, h2_local)
    a2, bb2 = bn_reduce(2)
    for wg in range(0, NW, 4):
        we = min(wg + 4, NW)
        yb = work.tile([128, 4 * 128], f32, tag="tnm")
        nfull = 0
        part = None
        for w in range(wg, we):
            wdst = min(WIN, NPC - w * WIN)
            j0 = (w - wg) * 128
            ocm = work.tile([128, 128], f32, tag="ocm")
            nc.scalar.activation(
                out=ocm[:], in_=zres[:, w * 128:(w + 1) * 128],
                func=Act.Identity, scale=a2[:, 0:1], bias=bb2[:, 0:1])
            tp = psum_misc.tile([128, 512], f32, tag="pm")
            nc.tensor.transpose(tp[:, :128], ocm[:, :], ident[:])
            nc.vector.tensor_copy(out=yb[:wdst, j0:j0 + 128],
                                  in_=tp[:wdst, :128])
            if wdst == 128:
                nfull += 1
            else:
                part = (w, j0, wdst)
        if nfull and cfg.BATCH_IO:
            nc.sync.dma_start(
                y[wg * 128:wg * 128 + nfull * 128, :].rearrange(
                    "(t p) d -> p t d", p=128),
                yb[:, :nfull * 128].rearrange("p (t d) -> p t d", d=128))
        elif nfull:
            for t in range(nfull):
                nc.sync.dma_start(
                    y[(wg + t) * 128:(wg + t + 1) * 128, :],
                    yb[:, t * 128:(t + 1) * 128])
        if part is not None:
            w, j0, wdst = part
            nc.sync.dma_start(y[w * 128:w * 128 + wdst, :],
                              yb[:wdst, j0:j0 + 128])


def build_program(cfg: Cfg, sched: Sched):
    """Create Bass program; returns nc."""
    import concourse.bacc as bacc
    import concourse.mybir as mybir
    from concourse.tile import TileContext
    _apply_tile_patch()

    f32 = mybir.dt.float32
    nc = bacc.Bacc(num_devices=cfg.CORES, num_swdge_queues=cfg.NQ,
                   dynamic_dma_scratch_size=cfg.DMASCRATCH)
    CT = sched.CT
    NCALLS = cfg.NSB * cfg.NSEG
    tensors = {
        "x_shard": nc.dram_tensor("x_shard", [cfg.NPC, cfg.D], f32,
                                  kind="ExternalInput"),
        "consts": nc.dram_tensor("consts", [128, 129], f32,
                                 kind="ExternalInput"),
        "meta": nc.dram_tensor("meta", [128, 8 * CT], mybir.dt.int16,
                               kind="ExternalInput"),
        "ohmat": nc.dram_tensor("ohmat", [128, 128 * (CT + cfg.NW)],
                                mybir.dt.float16, kind="ExternalInput"),
        "ccnt": nc.dram_tensor(
            "ccnt",
            [1, sum(-(-n // (cfg.GSPLIT or 10 ** 9))
                    for n in sched.n_chunks_call.values())],
            mybir.dt.int32, kind="ExternalInput"),
        "W1": nc.dram_tensor("W1", [128, 128], f32, kind="ExternalInput"),
        "W2": nc.dram_tensor("W2", [128, 128], f32, kind="ExternalInput"),
        "b1": nc.dram_tensor("b1", [128, 1], f32, kind="ExternalInput"),
        "g1": nc.dram_tensor("g1", [128, 1], f32, kind="ExternalInput"),
        "be1": nc.dram_tensor("be1", [128, 1], f32, kind="ExternalInput"),
        "b2": nc.dram_tensor("b2", [128, 1], f32, kind="ExternalInput"),
        "g2": nc.dram_tensor("g2", [128, 1], f32, kind="ExternalInput"),
        "be2": nc.dram_tensor("be2", [128, 1], f32, kind="ExternalInput"),
        "y": nc.dram_tensor("y", [cfg.NPC, cfg.D], f32, kind="ExternalOutput"),
    }
    with TileContext(nc) as tc:
        build(nc, tc, cfg, sched, tensors)
        tc._gnn_exitstack.close()
    if not nc.is_finalized():
        nc.finalize()
    return nc


def make_consts():
    c = np.zeros((128, 129), np.float32)
    c[:, :128] = np.arange(128, dtype=np.float32)[None, :]
    c[:, 128] = np.arange(128, dtype=np.float32)
    return c


def kernel_run(inputs: dict, cfg: Cfg):
    """Full flow: prep -> build -> run on 8 cores -> assemble output."""
    import numpy as np
    from concourse.bass_utils import run_bass_kernel_spmd

    x = np.asarray(inputs["x"], np.float32)
    ei = np.asarray(inputs["edge_index"])
    ew = np.asarray(inputs["edge_weight"], np.float32)
    sched, per_core = prep(ei, ew, cfg)
    nc = build_program(cfg, sched)

    com = {
        "W1": np.ascontiguousarray(inputs["W1"], dtype=np.float32),
        "W2": np.ascontiguousarray(inputs["W2"], dtype=np.float32),
        "consts": make_consts(),
    }
    for nm in ("b1", "g1", "be1", "b2", "g2", "be2"):
        com[nm] = np.ascontiguousarray(
            np.asarray(inputs[nm], np.float32).reshape(128, 1))
    in_maps = []
    for ci in range(cfg.CORES):
        m = dict(com)
        m["x_shard"] = np.ascontiguousarray(x[ci * cfg.NPC:(ci + 1) * cfg.NPC])
        m["meta"] = per_core[ci]["meta"]
        m["ohmat"] = per_core[ci]["ohmat"]
        m["ccnt"] = per_core[ci]["ccnt"]
        in_maps.append(m)
    res = run_bass_kernel_spmd(nc, in_maps, core_ids=list(range(cfg.CORES)))
    out = np.concatenate([r["y"] for r in res.results], axis=0)
    return out, res


def kernel(**inputs) -> np.ndarray:
    # Hermetic entry point: the GNN_* env vars are dev hooks (stage
    # truncation, config overrides) — never let an inherited environment
    # change what the harness builds.
    import os
    for k in list(os.environ):
        if k.startswith("GNN_"):
            del os.environ[k]
    cfg = Cfg()
    out, _ = kernel_run(inputs, cfg)
    return out

